# revision 1
# baseline (speedup 1.0000x reference)
"""KDE2D Trainium2 Bass kernel.

Reference computation (per (b,t) pair, B=16, T=64, N=512, grid 128x128):
  standardize points (mean/std ddof=1 over N), then
  density[gx,gy] = norm * sum_n exp(-c*(xg[gx]-x_n)^2) * exp(-c*(yg[gy]-y_n)^2)
  with c = 1/(2 h^2), norm = 1/(2 pi h^2).

Kernel strategy (data-parallel over the 1024 (b,t) pairs, 128 per core):
  exp(-c(g - x)^2) = [e^{-c g^2 + K}] * exp(2c*x*g - c*x^2 - K)
  The second factor is built per (bt, n-chunk) tile [n=128 part, g=128 free]
  with ONE ScalarE activation op: Exp(scale_p * GXROW + bias_p), where
  scale_p = 2c*x_p and bias_p = -c*x_p^2 - K are per-partition operands.
  bf16 tiles feed a 4-chunk accumulating PE matmul (contract n=512) into
  PSUM; the rank-1 factor beta_gx*beta_gy (which also carries norm and
  e^{2K}) is applied by one DVE scalar_tensor_tensor op, then DMA out.
  K keeps bf16/psum values in range (max product exponent 2*c*25 - 2K).
"""

import math

import numpy as np

import concourse.bass as bass
import concourse.bacc as bacc
import concourse.mybir as mybir
from concourse import tile
from concourse.bass_utils import run_bass_kernel_spmd

B, T, N, GRID = 16, 64, 512, 128
NCORES = 8
BT_PER_CORE = (B * T) // NCORES  # 128
NCHUNK = N // 128  # 4

F32 = mybir.dt.float32
BF16 = mybir.dt.bfloat16

_CACHE = {}


def _build(bw: float):
    h = float(bw)
    c = 1.0 / (2.0 * h * h)
    norm = 1.0 / (2.0 * math.pi * h * h)
    gmax = 5.0
    K = c * gmax * gmax / 2.0  # per-side exponent shift

    nc = bacc.Bacc("TRN2", target_bir_lowering=False)
    a_ext = nc.declare_dram_parameter("a", [BT_PER_CORE, N, 2], F32, isOutput=False)
    gx_ext = nc.declare_dram_parameter("gxrow", [128, GRID], F32, isOutput=False)
    idt_ext = nc.declare_dram_parameter("idt", [128, 128], F32, isOutput=False)
    bx_ext = nc.declare_dram_parameter("betax", [128, 1], F32, isOutput=False)
    by_ext = nc.declare_dram_parameter("betay", [128, GRID], F32, isOutput=False)
    out_ext = nc.declare_dram_parameter(
        "out", [BT_PER_CORE, GRID, GRID], F32, isOutput=True
    )

    AT = mybir.ActivationFunctionType
    OP = mybir.AluOpType

    with tile.TileContext(nc) as tc:
        with (
            tc.tile_pool(name="const", bufs=1) as cpool,
            tc.tile_pool(name="stats", bufs=1) as spool,
            tc.tile_pool(name="work", bufs=3) as wpool,
            tc.tile_pool(name="exy", bufs=12) as epool,
            tc.tile_pool(name="psum", bufs=6, space="PSUM") as ppool,
            tc.tile_pool(name="psumT", bufs=2, space="PSUM") as tpool,
            tc.tile_pool(name="outp", bufs=6) as opool,
        ):
            gx_sb = cpool.tile([128, GRID], F32, tag="gx")
            idt_sb = cpool.tile([128, 128], F32, tag="idt")
            bx_sb = cpool.tile([128, 1], F32, tag="bx")
            by_sb = cpool.tile([128, GRID], F32, tag="by")
            nc.sync.dma_start(gx_sb[:], gx_ext[:])
            nc.sync.dma_start(idt_sb[:], idt_ext[:])
            nc.sync.dma_start(bx_sb[:], bx_ext[:])
            nc.sync.dma_start(by_sb[:], by_ext[:])

            # ---- load points contiguously: [bt(128 part), n, ch] ----
            a_all = spool.tile([128, N, 2], F32, tag="a")
            nc.sync.dma_start(a_all[:], a_ext[:])
            x_sb = a_all[:, :, 0]
            y_sb = a_all[:, :, 1]

            # ---- per-bt stats and derived scale/bias arrays (layout [bt, n]) ----
            # sx = 2c * (x-mean)*invsd ; biasx = -c*((x-mean)*invsd)^2 - K
            derived = {}
            for ch, src in (("x", x_sb), ("y", y_sb)):
                s1 = spool.tile([128, 1], F32, tag=f"s1{ch}")
                s2 = spool.tile([128, 1], F32, tag=f"s2{ch}")
                sq = wpool.tile([128, N], F32, tag="sq")
                nc.vector.tensor_reduce(s1[:], src, mybir.AxisListType.X, OP.add)
                nc.vector.tensor_tensor(sq[:], src, src, OP.mult)
                nc.vector.tensor_reduce(s2[:], sq[:], mybir.AxisListType.X, OP.add)
                mean = spool.tile([128, 1], F32, tag=f"mean{ch}")
                nc.vector.tensor_scalar_mul(mean[:], s1[:], 1.0 / N)
                m2 = spool.tile([128, 1], F32, tag=f"m2{ch}")
                nc.vector.tensor_tensor(m2[:], mean[:], mean[:], OP.mult)
                var = spool.tile([128, 1], F32, tag=f"var{ch}")
                # var = (s2 - N*m2) / (N-1)
                nc.vector.scalar_tensor_tensor(
                    var[:], m2[:], -float(N), s2[:], OP.mult, OP.add
                )
                nc.vector.tensor_scalar_mul(var[:], var[:], 1.0 / (N - 1))
                sd = spool.tile([128, 1], F32, tag=f"sd{ch}")
                nc.scalar.activation(sd[:], var[:], AT.Sqrt)
                invsd = spool.tile([128, 1], F32, tag=f"invsd{ch}")
                nc.vector.reciprocal(invsd[:], sd[:])

                # xt = (x - mean) * invsd  (two tensor_scalar ops)
                xt = wpool.tile([128, N], F32, tag=f"xt{ch}")
                nc.vector.tensor_scalar(
                    xt[:], src, mean[:, 0:1], None, OP.subtract
                )
                nc.vector.tensor_scalar(
                    xt[:], xt[:], invsd[:, 0:1], None, OP.mult
                )
                # scale array: 2c * xt
                sc = wpool.tile([128, N], F32, tag=f"sc{ch}")
                nc.vector.tensor_scalar_mul(sc[:], xt[:], 2.0 * c)
                # bias array: -c*xt^2 - K
                bi = wpool.tile([128, N], F32, tag=f"bi{ch}")
                nc.vector.tensor_tensor(bi[:], xt[:], xt[:], OP.mult)
                nc.vector.tensor_scalar(bi[:], bi[:], -c, -K, OP.mult, OP.add)
                derived[ch] = (sc, bi)

            # ---- transpose derived arrays to [n(part), bt] via PE ----
            # Matmult instructions only tolerate ONE sync wait in walrus
            # codegen, so absorb the idt/gx DMA ticks into PE/ACT clocks
            # with dummy ops before the real transposes run.
            dummy_pt = tpool.tile([128, 128], F32, tag="pt")
            nc.tensor.transpose(dummy_pt[:], idt_sb[:], idt_sb[:])
            gx_probe = spool.tile([128, 1], F32, tag="gxprobe")
            nc.scalar.activation(gx_probe[:], gx_sb[:, 0:1], AT.Copy)
            # arrT[cc][:, bt] columns feed activation scale/bias operands.
            trans = {}
            for name, arr in (
                ("scx", derived["x"][0]),
                ("bix", derived["x"][1]),
                ("scy", derived["y"][0]),
                ("biy", derived["y"][1]),
            ):
                tiles = []
                for cc in range(NCHUNK):
                    pt = tpool.tile([128, 128], F32, tag="pt")
                    nc.tensor.transpose(
                        pt[:], arr[:, cc * 128 : (cc + 1) * 128], idt_sb[:]
                    )
                    st = cpool.tile([128, 128], F32, tag=f"T{name}{cc}")
                    nc.vector.tensor_copy(st[:], pt[:])
                    tiles.append(st)
                trans[name] = tiles

            # ---- main loop: one (bt) per iteration ----
            for bt in range(BT_PER_CORE):
                ps = ppool.tile([128, GRID], F32, tag="ps")
                exs, eys = [], []
                for cc in range(NCHUNK):
                    ex = epool.tile([128, GRID], BF16, tag="ex")
                    ey = epool.tile([128, GRID], BF16, tag="ey")
                    nc.scalar.activation(
                        ex[:], gx_sb[:], AT.Exp,
                        bias=trans["bix"][cc][:, bt : bt + 1],
                        scale=trans["scx"][cc][:, bt : bt + 1],
                    )
                    nc.scalar.activation(
                        ey[:], gx_sb[:], AT.Exp,
                        bias=trans["biy"][cc][:, bt : bt + 1],
                        scale=trans["scy"][cc][:, bt : bt + 1],
                    )
                    exs.append(ex)
                    eys.append(ey)
                for cc in range(NCHUNK):
                    nc.tensor.matmul(
                        ps[:], exs[cc][:], eys[cc][:],
                        start=(cc == 0), stop=(cc == NCHUNK - 1),
                    )
                ob = opool.tile([128, GRID], F32, tag="ob")
                # out = (psum * betax_p) * betay_row  (one DVE op)
                nc.vector.scalar_tensor_tensor(
                    ob[:], ps[:], bx_sb[:, 0:1], by_sb[:], OP.mult, OP.mult
                )
                nc.sync.dma_start(out_ext[bt], ob[:])

    if not nc.is_finalized():
        nc.finalize()
    return nc


def _consts(bw: float):
    h = float(bw)
    c = 1.0 / (2.0 * h * h)
    norm = 1.0 / (2.0 * math.pi * h * h)
    gmax = 5.0
    K = c * gmax * gmax / 2.0
    xg = np.linspace(-5.0, 5.0, GRID, dtype=np.float64)
    gxrow = np.broadcast_to(xg.astype(np.float32), (128, GRID)).copy()
    idt = np.eye(128, dtype=np.float32)
    betax = np.exp(K - c * xg * xg).astype(np.float32).reshape(GRID, 1)
    betay = (norm * np.exp(K - c * xg * xg)).astype(np.float32)
    betay = np.broadcast_to(betay, (128, GRID)).copy()
    return gxrow, idt, betax, betay


def kernel(A: np.ndarray, bandwidth: np.ndarray) -> np.ndarray:
    A = np.asarray(A, dtype=np.float32)
    bw = float(np.asarray(bandwidth))
    key = round(bw, 9)
    if key not in _CACHE:
        _CACHE[key] = _build(bw)
    nc = _CACHE[key]

    gxrow, idt, betax, betay = _consts(bw)
    a_flat = A.reshape(B * T, N, 2)
    in_maps = []
    for i in range(NCORES):
        in_maps.append(
            {
                "a": np.ascontiguousarray(
                    a_flat[i * BT_PER_CORE : (i + 1) * BT_PER_CORE]
                ),
                "gxrow": gxrow,
                "idt": idt,
                "betax": betax,
                "betay": betay,
            }
        )
    res = run_bass_kernel_spmd(nc, in_maps, core_ids=list(range(NCORES)))
    outs = [res.results[i]["out"] for i in range(NCORES)]
    return np.concatenate(outs, axis=0).reshape(B, T, GRID, GRID)


if __name__ == "__main__":
    A = np.random.randn(B, T, N, 2).astype(np.float32)
    out = kernel(A, np.float32(0.5))
    print(out.shape, out.dtype, float(out.max()))



# revision 4
# speedup vs baseline: 2.6086x; 2.6086x over previous
"""KDE2D Trainium2 Bass kernel — Fourier (trig-moment) factorization.

Reference (per (b,t), B=16, T=64, N=512, grid 128x128, bandwidth h):
  standardize points (mean/std ddof=1 over N), then
  density[g,h] = 1/(2 pi h^2) * sum_n exp(-(xg-x_n)^2/2h^2) * exp(-(yg-y_n)^2/2h^2)
              = sum_n phi(xg - x_n) * phi(yg - y_n),   phi = 1D-normalized Gaussian.

Kernel idea: periodize phi with period L and truncate its Fourier series at K
harmonics.  With theta = 2*pi*x/L and the D = 2K+1 feature vector
v(x) = [1, cos(j*theta), sin(j*theta)]_{j=1..K}:
  phi(g - x) ~= sum_d U[d, g] * v_d(x)       (U solved host-side by lstsq)
  density    = Ux^T M Uy,   M[d, e] = sum_n v_d(x_n) v_e(y_n).
Per (b,t) the device work collapses to tiny matmuls:
  M2 = Vy^T Vx   (PE, contract n in 4 chunks of 128)
  Z  = M2^T-contract: lhsT=M2[e,d], rhs=U  -> Z[d, h]
  D  = lhsT=U[d, g] (const stationary), rhs=Z -> density[g, h]
V tiles [n, D] are built once per core for all 128 bt via half-angle Sin
seeds (ACT) + Chebyshev recurrences (DVE/Pool) in fp16 — no per-point exp.
Output is written f16 and upcast on host (halves output DMA traffic).

Validated in numpy: K=8, L=11 gives rel-Frobenius ~3e-3 (gate 2e-2).
"""

import math

import numpy as np

import concourse.bass as bass
import concourse.bacc as bacc
import concourse.mybir as mybir
from concourse import tile
from concourse.bass_utils import run_bass_kernel_spmd

B, T, N, GRID = 16, 64, 512, 128
NCORES = 8
BT_PER_CORE = (B * T) // NCORES  # 128
NCHUNK = N // 128  # 4

KHARM = 8            # Fourier harmonics
LPER = 11.0          # periodization length
DDIM = 2 * KHARM + 1  # 17 feature dims

F32 = mybir.dt.float32
F16 = mybir.dt.float16

_CACHE = {}


def _build(bw: float):
    AT = mybir.ActivationFunctionType
    OP = mybir.AluOpType

    nc = bacc.Bacc("TRN2", target_bir_lowering=False)
    a_ext = nc.declare_dram_parameter("a", [BT_PER_CORE, N, 2], F32, isOutput=False)
    idt_ext = nc.declare_dram_parameter("idt", [128, 128], F32, isOutput=False)
    u_ext = nc.declare_dram_parameter("umat", [DDIM, GRID], F16, isOutput=False)
    out_ext = nc.declare_dram_parameter(
        "out", [BT_PER_CORE, GRID, GRID], F16, isOutput=True
    )

    two_pi_over_l = 2.0 * math.pi / LPER

    with tile.TileContext(nc) as tc:
        with (
            tc.tile_pool(name="const", bufs=1) as cpool,
            tc.tile_pool(name="stats", bufs=1) as spool,
            tc.tile_pool(name="vpool", bufs=1) as vpool,
            tc.tile_pool(name="work", bufs=2) as wpool,
            tc.tile_pool(name="ptr", bufs=1, space="PSUM") as trpool,
            tc.tile_pool(name="pm", bufs=2, space="PSUM") as mpool,
            tc.tile_pool(name="pz", bufs=2, space="PSUM") as zpool,
            tc.tile_pool(name="pd", bufs=2, space="PSUM") as dpool,
            tc.tile_pool(name="msb", bufs=2) as msbpool,
            tc.tile_pool(name="zsb", bufs=2) as zsbpool,
            tc.tile_pool(name="dsb", bufs=2) as dsbpool,
        ):
            idt_sb = cpool.tile([128, 128], F32, tag="idt")
            u_sb = cpool.tile([DDIM, GRID], F16, tag="umat")
            halfpi = cpool.tile([128, 1], F32, tag="halfpi")
            nc.sync.dma_start(idt_sb[:], idt_ext[:])
            nc.sync.dma_start(u_sb[:], u_ext[:])
            nc.vector.memset(halfpi[:], math.pi / 2.0)

            a_all = spool.tile([128, N, 2], F32, tag="a")
            nc.sync.dma_start(a_all[:], a_ext[:])

            # ---- per-bt stats; theta = (x - mean) * invsd * (2 pi / L) ----
            # layout [bt(128 part), n(512 free)]
            theta = {}
            for ch, ci in (("x", 0), ("y", 1)):
                src = a_all[:, :, ci]
                s1 = spool.tile([128, 1], F32, tag=f"s1{ch}")
                s2 = spool.tile([128, 1], F32, tag=f"s2{ch}")
                sq = wpool.tile([128, N], F32, tag=f"sq{ch}")
                nc.vector.tensor_reduce(s1[:], src, mybir.AxisListType.X, OP.add)
                nc.vector.tensor_tensor(sq[:], src, src, OP.mult)
                nc.vector.tensor_reduce(s2[:], sq[:], mybir.AxisListType.X, OP.add)
                mean = spool.tile([128, 1], F32, tag=f"mean{ch}")
                nc.vector.tensor_scalar_mul(mean[:], s1[:], 1.0 / N)
                m2 = spool.tile([128, 1], F32, tag=f"m2{ch}")
                nc.vector.tensor_tensor(m2[:], mean[:], mean[:], OP.mult)
                var = spool.tile([128, 1], F32, tag=f"var{ch}")
                nc.vector.scalar_tensor_tensor(
                    var[:], m2[:], -float(N), s2[:], OP.mult, OP.add
                )
                nc.vector.tensor_scalar_mul(var[:], var[:], 1.0 / (N - 1))
                sd = spool.tile([128, 1], F32, tag=f"sd{ch}")
                nc.scalar.activation(sd[:], var[:], AT.Sqrt)
                invsd = spool.tile([128, 1], F32, tag=f"invsd{ch}")
                nc.vector.reciprocal(invsd[:], sd[:])
                alph = spool.tile([128, 1], F32, tag=f"alph{ch}")
                nc.vector.tensor_scalar_mul(alph[:], invsd[:], two_pi_over_l)
                th = spool.tile([128, N], F32, tag=f"th{ch}")
                # (x - mean) * alpha in one two-stage tensor_scalar
                nc.vector.tensor_scalar(
                    th[:], src, mean[:, 0:1], alph[:, 0:1], OP.subtract, OP.mult
                )
                theta[ch] = th

            # ---- transpose theta to [n(128 part), (chunk, bt)] ----
            thT = {}
            for ch in ("x", "y"):
                pt = trpool.tile([128, NCHUNK, 128], F32, tag=f"pt{ch}")
                for cc in range(NCHUNK):
                    nc.tensor.transpose(
                        pt[:, cc, :], theta[ch][:, cc * 128 : (cc + 1) * 128], idt_sb[:]
                    )
                tt = vpool.tile([128, NCHUNK, 128], F32, tag=f"thT{ch}")
                nc.vector.tensor_copy(tt[:], pt[:])
                thT[ch] = tt

            # ---- V tiles [n part, d, chunk, bt] f16 via seeds + Chebyshev ----
            # v_0 = 1, v_j = cos(j th), v_{K+j} = sin(j th)
            V = {}
            for ch in ("x", "y"):
                v = vpool.tile([128, DDIM, NCHUNK, 128], F16, tag=f"V{ch}")
                nc.vector.memset(v[:, 0], 1.0)
                hs = wpool.tile([128, NCHUNK, 128], F16, tag=f"hs{ch}")
                hc = wpool.tile([128, NCHUNK, 128], F16, tag=f"hc{ch}")
                # half-angle seeds keep Sin args within [-pi, pi]
                nc.scalar.activation(hs[:], thT[ch][:], AT.Sin, scale=0.5)
                nc.scalar.activation(
                    hc[:], thT[ch][:], AT.Sin, bias=halfpi[:, 0:1], scale=0.5
                )
                # sin th = 2 hs hc ; cos th = 1 - 2 hs^2
                prod = wpool.tile([128, NCHUNK, 128], F16, tag=f"pr{ch}")
                nc.vector.tensor_tensor(prod[:], hs[:], hc[:], OP.mult)
                nc.vector.tensor_scalar(
                    v[:, KHARM + 1], prod[:], 2.0, None, OP.mult
                )
                nc.vector.tensor_tensor(prod[:], hs[:], hs[:], OP.mult)
                nc.vector.tensor_scalar(
                    v[:, 1], prod[:], -2.0, 1.0, OP.mult, OP.add
                )
                t2 = wpool.tile([128, NCHUNK, 128], F16, tag=f"t2{ch}")
                nc.vector.tensor_scalar(t2[:], v[:, 1], 2.0, None, OP.mult)
                V[ch] = (v, t2)

            # Chebyshev: v_{j} = t2 * v_{j-1} - v_{j-2} for both cos and sin
            # chains; split the multiply/subtract across DVE and Pool to
            # balance engine load.
            for j in range(2, KHARM + 1):
                for ch in ("x", "y"):
                    v, t2 = V[ch]
                    for base in (0, KHARM):
                        # cos chain base 0 (v1=cos); sin chain base KHARM
                        jm1 = base + j - 1 if base else j - 1
                        jm2 = base + j - 2 if base else j - 2
                        if base and j == 2:
                            jm2 = 0  # sin_0 = 0 -> v_{K+2} = t2*v_{K+1} - 0
                        dst = base + j
                        u = wpool.tile([128, NCHUNK, 128], F16, tag=f"u{ch}{base}")
                        eng = nc.gpsimd if (j % 4 == 0 and base == KHARM) else nc.vector
                        eng.tensor_tensor(u[:], t2[:], v[:, jm1], OP.mult)
                        if base and j == 2:
                            # v_{K+2} = u  (sin_0 term is zero)
                            nc.vector.tensor_copy(v[:, dst], u[:])
                        else:
                            eng2 = nc.gpsimd if (j % 4 == 2 and base == 0) else nc.vector
                            eng2.tensor_tensor(v[:, dst], u[:], v[:, jm2], OP.subtract)

            vx, _ = V["x"]
            vy, _ = V["y"]

            # ---- main loop: groups of 4 bt ----
            NG = BT_PER_CORE // 4
            for gi in range(NG):
                mps = mpool.tile([DDIM, 4, DDIM], F32, tag="mps")
                for i in range(4):
                    bt = gi * 4 + i
                    for cc in range(NCHUNK):
                        # M2[e,d] = sum_n Vy[n,e] Vx[n,d]
                        nc.tensor.matmul(
                            mps[:, i, :],
                            vy[:, :, cc, bt],
                            vx[:, :, cc, bt],
                            start=(cc == 0),
                            stop=(cc == NCHUNK - 1),
                        )
                msb = msbpool.tile([DDIM, 4, DDIM], F16, tag="msb")
                nc.vector.tensor_copy(msb[:], mps[:])

                zps = zpool.tile([DDIM, 4, GRID], F32, tag="zps")
                for i in range(4):
                    # Z[d, h] = sum_e M2[e, d] U[e, h]
                    nc.tensor.matmul(
                        zps[:, i, :], msb[:, i, :], u_sb[:], start=True, stop=True
                    )
                zsb = zsbpool.tile([DDIM, 4, GRID], F16, tag="zsb")
                nc.scalar.activation(zsb[:], zps[:], AT.Copy)

                dps = dpool.tile([128, 4, GRID], F32, tag="dps")
                for i in range(4):
                    # density[g, h] = sum_d U[d, g] Z[d, h]
                    nc.tensor.matmul(
                        dps[:, i, :], u_sb[:], zsb[:, i, :], start=True, stop=True
                    )
                dsb = dsbpool.tile([128, 4, GRID], F16, tag="dsb")
                if gi % 2 == 0:
                    nc.vector.tensor_copy(dsb[:], dps[:])
                else:
                    nc.scalar.activation(dsb[:], dps[:], AT.Copy)
                for i in range(4):
                    nc.sync.dma_start(out_ext[gi * 4 + i], dsb[:, i, :])

    if not nc.is_finalized():
        nc.finalize()
    return nc


def _consts(bw: float):
    h = float(bw)
    g = np.linspace(-5.0, 5.0, GRID)
    xs = np.linspace(-5.1, 5.1, 4001)
    th = 2.0 * np.pi * xs / LPER
    cols = (
        [np.ones_like(th)]
        + [np.cos(j * th) for j in range(1, KHARM + 1)]
        + [np.sin(j * th) for j in range(1, KHARM + 1)]
    )
    Phi = np.stack(cols, axis=-1)  # [S, D]
    Tgt = np.exp(-((g[None, :] - xs[:, None]) ** 2) / (2.0 * h * h)) / (
        np.sqrt(2.0 * np.pi) * h
    )
    AtA = Phi.T @ Phi + 1e-7 * len(xs) * np.eye(DDIM)
    U = np.linalg.solve(AtA, Phi.T @ Tgt)  # [D, G]
    idt = np.eye(128, dtype=np.float32)
    return idt, U.astype(np.float16)


def kernel(A: np.ndarray, bandwidth: np.ndarray) -> np.ndarray:
    A = np.asarray(A, dtype=np.float32)
    bw = float(np.asarray(bandwidth))
    key = round(bw, 9)
    if key not in _CACHE:
        _CACHE[key] = _build(bw)
    nc = _CACHE[key]

    idt, umat = _consts(bw)
    a_flat = A.reshape(B * T, N, 2)
    in_maps = []
    for i in range(NCORES):
        in_maps.append(
            {
                "a": np.ascontiguousarray(
                    a_flat[i * BT_PER_CORE : (i + 1) * BT_PER_CORE]
                ),
                "idt": idt,
                "umat": umat,
            }
        )
    res = run_bass_kernel_spmd(nc, in_maps, core_ids=list(range(NCORES)))
    outs = [res.results[i]["out"] for i in range(NCORES)]
    return (
        np.concatenate(outs, axis=0).astype(np.float32).reshape(B, T, GRID, GRID)
    )


if __name__ == "__main__":
    Arand = np.random.randn(B, T, N, 2).astype(np.float32)
    out = kernel(Arand, np.float32(0.5))
    print(out.shape, out.dtype, float(out.max()))


# revision 5
# speedup vs baseline: 4.1721x; 1.5993x over previous
"""KDE2D Trainium2 Bass kernel — Fourier (trig-moment) factorization.

Reference (per (b,t), B=16, T=64, N=512, grid 128x128, bandwidth h):
  standardize points (mean/std ddof=1 over N), then
  density[g,h] = 1/(2 pi h^2) * sum_n exp(-(xg-x_n)^2/2h^2) * exp(-(yg-y_n)^2/2h^2)
              = sum_n phi(xg - x_n) * phi(yg - y_n),   phi = 1D-normalized Gaussian.

Kernel idea: periodize phi with period L and truncate its Fourier series at K
harmonics.  With theta = 2*pi*x/L and the D = 2K+1 feature vector
v(x) = [1, cos(j*theta), sin(j*theta)]_{j=1..K}:
  phi(g - x) ~= sum_d U[d, g] * v_d(x)       (U solved host-side by lstsq)
  density    = Ux^T M Uy,   M[d, e] = sum_n v_d(x_n) v_e(y_n).
Per (b,t) the device work collapses to tiny matmuls:
  M2 = Vy^T Vx   (PE, contract n in 4 chunks of 128)
  Z  = M2^T-contract: lhsT=M2[e,d], rhs=U  -> Z[d, h]
  D  = lhsT=U[d, g] (const stationary), rhs=Z -> density[g, h]
V tiles [n, D] are built once per core for all 128 bt via half-angle Sin
seeds (ACT) + Chebyshev recurrences (DVE/Pool) in fp16 — no per-point exp.
Output is written f16 and upcast on host (halves output DMA traffic).

Validated in numpy: K=8, L=11 gives rel-Frobenius ~3e-3 (gate 2e-2).
"""

import math

import numpy as np

import concourse.bass as bass
import concourse.bacc as bacc
import concourse.mybir as mybir
from concourse import tile
from concourse.bass_utils import run_bass_kernel_spmd

B, T, N, GRID = 16, 64, 512, 128
NCORES = 8
BT_PER_CORE = (B * T) // NCORES  # 128
NCHUNK = N // 128  # 4

KHARM = 8            # Fourier harmonics
LPER = 11.0          # periodization length
DDIM = 2 * KHARM + 1  # 17 feature dims

F32 = mybir.dt.float32
F16 = mybir.dt.float16

_CACHE = {}


def _build(bw: float):
    AT = mybir.ActivationFunctionType
    OP = mybir.AluOpType

    nc = bacc.Bacc("TRN2", target_bir_lowering=False)
    a_ext = nc.declare_dram_parameter("a", [BT_PER_CORE, N, 2], F32, isOutput=False)
    idt_ext = nc.declare_dram_parameter("idt", [128, 128], F32, isOutput=False)
    u_ext = nc.declare_dram_parameter("umat", [DDIM, GRID], F16, isOutput=False)
    out_ext = nc.declare_dram_parameter(
        "out", [BT_PER_CORE, GRID, GRID], F16, isOutput=True
    )

    two_pi_over_l = 2.0 * math.pi / LPER

    with tile.TileContext(nc) as tc:
        with (
            tc.tile_pool(name="const", bufs=1) as cpool,
            tc.tile_pool(name="stats", bufs=1) as spool,
            tc.tile_pool(name="vpool", bufs=1) as vpool,
            tc.tile_pool(name="work", bufs=2) as wpool,
            tc.tile_pool(name="ptr", bufs=1, space="PSUM") as trpool,
            tc.tile_pool(name="pm", bufs=2, space="PSUM") as mpool,
            tc.tile_pool(name="pz", bufs=2, space="PSUM") as zpool,
            tc.tile_pool(name="pd", bufs=2, space="PSUM") as dpool,
            tc.tile_pool(name="msb", bufs=2) as msbpool,
            tc.tile_pool(name="zsb", bufs=2) as zsbpool,
            tc.tile_pool(name="dsb", bufs=2) as dsbpool,
        ):
            idt_sb = cpool.tile([128, 128], F32, tag="idt")
            u_sb = cpool.tile([DDIM, GRID], F16, tag="umat")
            halfpi = cpool.tile([128, 1], F32, tag="halfpi")
            nc.sync.dma_start(idt_sb[:], idt_ext[:])
            nc.sync.dma_start(u_sb[:], u_ext[:])
            nc.vector.memset(halfpi[:], math.pi / 2.0)

            a_all = spool.tile([128, N, 2], F32, tag="a")
            nc.sync.dma_start(a_all[:], a_ext[:])

            # ---- per-bt stats; theta = (x - mean) * invsd * (2 pi / L) ----
            # layout [bt(128 part), n(512 free)]
            theta = {}
            for ch, ci in (("x", 0), ("y", 1)):
                src = a_all[:, :, ci]
                s1 = spool.tile([128, 1], F32, tag=f"s1{ch}")
                s2 = spool.tile([128, 1], F32, tag=f"s2{ch}")
                sq = wpool.tile([128, N], F32, tag=f"sq{ch}")
                nc.vector.tensor_reduce(s1[:], src, mybir.AxisListType.X, OP.add)
                nc.vector.tensor_tensor(sq[:], src, src, OP.mult)
                nc.vector.tensor_reduce(s2[:], sq[:], mybir.AxisListType.X, OP.add)
                mean = spool.tile([128, 1], F32, tag=f"mean{ch}")
                nc.vector.tensor_scalar_mul(mean[:], s1[:], 1.0 / N)
                m2 = spool.tile([128, 1], F32, tag=f"m2{ch}")
                nc.vector.tensor_tensor(m2[:], mean[:], mean[:], OP.mult)
                var = spool.tile([128, 1], F32, tag=f"var{ch}")
                nc.vector.scalar_tensor_tensor(
                    var[:], m2[:], -float(N), s2[:], OP.mult, OP.add
                )
                nc.vector.tensor_scalar_mul(var[:], var[:], 1.0 / (N - 1))
                sd = spool.tile([128, 1], F32, tag=f"sd{ch}")
                nc.scalar.activation(sd[:], var[:], AT.Sqrt)
                invsd = spool.tile([128, 1], F32, tag=f"invsd{ch}")
                nc.vector.reciprocal(invsd[:], sd[:])
                alph = spool.tile([128, 1], F32, tag=f"alph{ch}")
                nc.vector.tensor_scalar_mul(alph[:], invsd[:], two_pi_over_l)
                th = spool.tile([128, N], F32, tag=f"th{ch}")
                # (x - mean) * alpha in one two-stage tensor_scalar
                nc.vector.tensor_scalar(
                    th[:], src, mean[:, 0:1], alph[:, 0:1], OP.subtract, OP.mult
                )
                theta[ch] = th

            # ---- transpose theta to [n(128 part), (chunk, bt)] ----
            thT = {}
            for ch in ("x", "y"):
                pt = trpool.tile([128, NCHUNK, 128], F32, tag=f"pt{ch}")
                for cc in range(NCHUNK):
                    nc.tensor.transpose(
                        pt[:, cc, :], theta[ch][:, cc * 128 : (cc + 1) * 128], idt_sb[:]
                    )
                tt = vpool.tile([128, NCHUNK, 128], F32, tag=f"thT{ch}")
                nc.vector.tensor_copy(tt[:], pt[:])
                thT[ch] = tt

            # ---- V tiles [n part, d, chunk, bt] f16 via seeds + Chebyshev ----
            # v_0 = 1, v_j = cos(j th), v_{K+j} = sin(j th)
            V = {}
            for ch in ("x", "y"):
                v = vpool.tile([128, DDIM, NCHUNK, 128], F16, tag=f"V{ch}")
                nc.vector.memset(v[:, 0], 1.0)
                hs = wpool.tile([128, NCHUNK, 128], F16, tag=f"hs{ch}")
                hc = wpool.tile([128, NCHUNK, 128], F16, tag=f"hc{ch}")
                # half-angle seeds keep Sin args within [-pi, pi]
                nc.scalar.activation(hs[:], thT[ch][:], AT.Sin, scale=0.5)
                nc.scalar.activation(
                    hc[:], thT[ch][:], AT.Sin, bias=halfpi[:, 0:1], scale=0.5
                )
                # sin th = 2 hs hc ; cos th = 1 - 2 hs^2
                prod = wpool.tile([128, NCHUNK, 128], F16, tag=f"pr{ch}")
                nc.vector.tensor_tensor(prod[:], hs[:], hc[:], OP.mult)
                nc.vector.tensor_scalar(
                    v[:, KHARM + 1], prod[:], 2.0, None, OP.mult
                )
                nc.vector.tensor_tensor(prod[:], hs[:], hs[:], OP.mult)
                nc.vector.tensor_scalar(
                    v[:, 1], prod[:], -2.0, 1.0, OP.mult, OP.add
                )
                t2 = wpool.tile([128, NCHUNK, 128], F16, tag=f"t2{ch}")
                nc.vector.tensor_scalar(t2[:], v[:, 1], 2.0, None, OP.mult)
                V[ch] = (v, t2)

            # Chebyshev: v_{j} = t2 * v_{j-1} - v_{j-2} for both cos and sin
            # chains; split the multiply/subtract across DVE and Pool to
            # balance engine load.
            for j in range(2, KHARM + 1):
                for ch in ("x", "y"):
                    v, t2 = V[ch]
                    for base in (0, KHARM):
                        # cos chain base 0 (v1=cos); sin chain base KHARM
                        jm1 = base + j - 1 if base else j - 1
                        jm2 = base + j - 2 if base else j - 2
                        if base and j == 2:
                            jm2 = 0  # sin_0 = 0 -> v_{K+2} = t2*v_{K+1} - 0
                        dst = base + j
                        u = wpool.tile([128, NCHUNK, 128], F16, tag=f"u{ch}{base}")
                        eng = nc.gpsimd if (j % 4 == 0 and base == KHARM) else nc.vector
                        eng.tensor_tensor(u[:], t2[:], v[:, jm1], OP.mult)
                        if base and j == 2:
                            # v_{K+2} = u  (sin_0 term is zero)
                            nc.vector.tensor_copy(v[:, dst], u[:])
                        else:
                            eng2 = nc.gpsimd if (j % 4 == 2 and base == 0) else nc.vector
                            eng2.tensor_tensor(v[:, dst], u[:], v[:, jm2], OP.subtract)

            vx, _ = V["x"]
            vy, _ = V["y"]

            # ---- main loop: groups of 4 bt; DMA out in 8-bt batches ----
            NG = BT_PER_CORE // 4
            dsb = None
            for gi in range(NG):
                mps = mpool.tile([DDIM, 4, DDIM], F32, tag="mps")
                for i in range(4):
                    bt = gi * 4 + i
                    for cc in range(NCHUNK):
                        # M2[e,d] = sum_n Vy[n,e] Vx[n,d]
                        nc.tensor.matmul(
                            mps[:, i, :],
                            vy[:, :, cc, bt],
                            vx[:, :, cc, bt],
                            start=(cc == 0),
                            stop=(cc == NCHUNK - 1),
                        )
                msb = msbpool.tile([DDIM, 4, DDIM], F16, tag="msb")
                nc.vector.tensor_copy(msb[:], mps[:])

                zps = zpool.tile([DDIM, 4, GRID], F32, tag="zps")
                for i in range(4):
                    # Z[d, h] = sum_e M2[e, d] U[e, h]
                    nc.tensor.matmul(
                        zps[:, i, :], msb[:, i, :], u_sb[:], start=True, stop=True
                    )
                zsb = zsbpool.tile([DDIM, 4, GRID], F16, tag="zsb")
                nc.scalar.activation(zsb[:], zps[:], AT.Copy)

                dps = dpool.tile([128, 4, GRID], F32, tag="dps")
                for i in range(4):
                    # density[g, h] = sum_d U[d, g] Z[d, h]
                    nc.tensor.matmul(
                        dps[:, i, :], u_sb[:], zsb[:, i, :], start=True, stop=True
                    )
                if gi % 2 == 0:
                    dsb = dsbpool.tile([128, 8, GRID], F16, tag="dsb")
                half = dsb[:, (gi % 2) * 4 : (gi % 2) * 4 + 4, :]
                if gi % 2 == 0:
                    nc.vector.tensor_copy(half, dps[:])
                else:
                    nc.scalar.activation(half, dps[:], AT.Copy)
                if gi % 2 == 1:
                    dst = out_ext[(gi - 1) * 4 : (gi + 1) * 4].transpose([1, 0, 2])
                    nc.sync.dma_start(dst, dsb[:])

    if not nc.is_finalized():
        nc.finalize()
    return nc


def _consts(bw: float):
    h = float(bw)
    g = np.linspace(-5.0, 5.0, GRID)
    xs = np.linspace(-5.1, 5.1, 4001)
    th = 2.0 * np.pi * xs / LPER
    cols = (
        [np.ones_like(th)]
        + [np.cos(j * th) for j in range(1, KHARM + 1)]
        + [np.sin(j * th) for j in range(1, KHARM + 1)]
    )
    Phi = np.stack(cols, axis=-1)  # [S, D]
    Tgt = np.exp(-((g[None, :] - xs[:, None]) ** 2) / (2.0 * h * h)) / (
        np.sqrt(2.0 * np.pi) * h
    )
    AtA = Phi.T @ Phi + 1e-7 * len(xs) * np.eye(DDIM)
    U = np.linalg.solve(AtA, Phi.T @ Tgt)  # [D, G]
    idt = np.eye(128, dtype=np.float32)
    return idt, U.astype(np.float16)


def kernel(A: np.ndarray, bandwidth: np.ndarray) -> np.ndarray:
    A = np.asarray(A, dtype=np.float32)
    bw = float(np.asarray(bandwidth))
    key = round(bw, 9)
    if key not in _CACHE:
        _CACHE[key] = _build(bw)
    nc = _CACHE[key]

    idt, umat = _consts(bw)
    a_flat = A.reshape(B * T, N, 2)
    in_maps = []
    for i in range(NCORES):
        in_maps.append(
            {
                "a": np.ascontiguousarray(
                    a_flat[i * BT_PER_CORE : (i + 1) * BT_PER_CORE]
                ),
                "idt": idt,
                "umat": umat,
            }
        )
    res = run_bass_kernel_spmd(nc, in_maps, core_ids=list(range(NCORES)))
    outs = [res.results[i]["out"] for i in range(NCORES)]
    return (
        np.concatenate(outs, axis=0).astype(np.float32).reshape(B, T, GRID, GRID)
    )


if __name__ == "__main__":
    Arand = np.random.randn(B, T, N, 2).astype(np.float32)
    out = kernel(Arand, np.float32(0.5))
    print(out.shape, out.dtype, float(out.max()))
